# revision 1
# baseline (speedup 1.0000x reference)
"""Trainium2 Bass kernel for the dense branch-MLP problem (fp8 DoubleRow).

Computes: out[b,o] = sum_n relu((s[b,:] - v[n,:]) @ W[n].T + bias[n])[o]
with B=1024, N=64, D=512, OUT=2048 in fp32, graded at rel_absmax < 2e-2.

Sharding: expert-style across the N=64 branch axis -> 8 branches per core;
the host sums the 8 partial [B, OUT] results, descales, and adds the fold
constant (below).

Math restructure (host side):
  (s - v_n) @ W_n^T + b_n == s @ W_n^T + c_n,  c_n = b_n - v_n @ W_n^T
and for DVE-drained units the relu is rewritten relu(x+c) = max(x,-c) + c
with the dangling +c summed into a host-side per-o fold vector. s and
(16*W) are quantized to fp8 e4m3 on the host.

Schedule: psum partitions = o, BRANCH-OUTER loop. Per unit (nl, ot):
4 fp8 DoubleRow matmuls (0.5 cyc/row, 2 k-tiles each) -> psum [128,1024];
drain either fused on DVE (scalar_tensor_tensor: acc = (psum MAX -c) ADD
acc) or ACT relu(bias=c) -> tmp with the add on DVE tensor_tensor /
gpsimd accumulating DMA / gpsimd tensor_tensor. Each acc[ot] chain has
one link per branch, 16 units apart, so add latency never binds; out
DMAs trickle per-ot during the last branch pass. No bias matmuls: PE
does only the 512 branch DoubleRows (~59us) and the drain engines bind
at ~80us. Cost-model 89584ns (baseline 235646, 2.63x); HW rel_absmax 1.363e-2.
"""
import numpy as np
import ml_dtypes
import concourse.bacc as bacc
import concourse.mybir as mybir
import concourse.tile as tile
from concourse.bass_utils import run_bass_kernel_spmd

B, N, D, OUT = 1024, 64, 512, 2048
N_CORES = 8
NL, DC, OT, NQ = 8, 4, 16, 4
AW = 16.0
F32, BF16, F8 = mybir.dt.float32, mybir.dt.bfloat16, mybir.dt.float8e4
RELU = mybir.ActivationFunctionType.Relu
DR = mybir.MatmulPerfMode.DoubleRow
ADD, MAX = mybir.AluOpType.add, mybir.AluOpType.max

N_D_UNITS = 59


def _spread(total, picks):
    return [(i * picks) // total != ((i - 1) * picks) // total for i in range(total)]


_DMASK = _spread(128, N_D_UNITS)
_FORM = {}
for _nl in range(NL):
    for _ot in range(OT):
        _FORM[(_nl, _ot)] = "D" if _DMASK[_nl * OT + _ot] else "A"
_FORM[(7, 14)] = _FORM[(7, 15)] = "A"


def _a_add_path(nl, ot, k):
    if nl <= 1:
        return "P" if ot % 2 else "V"
    if nl >= NL - 1:
        return "V"
    return "M"


_cache = {}


def build(repeat: int = 1):
    if repeat in _cache:
        return _cache[repeat]
    nc = bacc.Bacc("TRN2", target_bir_lowering=False, debug=False, num_devices=N_CORES)
    wt_d = nc.dram_tensor("wt", [NL, NQ, 128, DC * 512], F8, kind="ExternalInput").ap()
    st_d = nc.dram_tensor("st", [128, DC * B], F8, kind="ExternalInput").ap()
    cb_d = nc.dram_tensor("cb", [128, NL * OT], F32, kind="ExternalInput").ap()
    ncb_d = nc.dram_tensor("ncb", [128, NL * OT], F32, kind="ExternalInput").ap()
    out_d = nc.dram_tensor("out", [128, OT * B], BF16, kind="ExternalOutput").ap()

    with tile.TileContext(nc) as tc:
        with (
            tc.tile_pool(name="const", bufs=1) as const_pool,
            tc.tile_pool(name="acc", bufs=1) as acc_pool,
            tc.tile_pool(name="tmp", bufs=4) as tmp_pool,
            tc.tile_pool(name="wt", bufs=2) as wt_pool,
            tc.tile_pool(name="psum", bufs=4, space="PSUM") as psum_pool,
        ):
            cb = const_pool.tile([128, NL * OT], F32, name="cb")
            ncbt = const_pool.tile([128, NL * OT], F32, name="ncb")
            nc.sync.dma_start(cb[:], cb_d[:])
            nc.sync.dma_start(ncbt[:], ncb_d[:])
            st = const_pool.tile([128, DC, B], F8, name="st")
            st_d3 = st_d.rearrange("p (c b) -> p c b", c=DC)

            def new_wt():
                return wt_pool.tile([128, DC, OUT], F8, name="wt_t", tag="wt_t")

            def wt_chunk_dma(wt, nl, q):
                wd3 = wt_d[nl, q].rearrange("p (c o) -> p c o", c=DC)
                nc.sync.dma_start(wt[:, :, q * 512 : q * 512 + 512], wd3)

            nc.sync.dma_start(st[:, 0:2, :], st_d3[:, 0:2, :])
            wt0 = new_wt()
            wt_chunk_dma(wt0, 0, 0)
            nc.sync.dma_start(st[:, 2:4, :], st_d3[:, 2:4, :])
            for q in range(1, NQ):
                wt_chunk_dma(wt0, 0, q)

            scr = const_pool.tile([128, 128], BF16, name="scr")
            nc.vector.memset(scr[:], 0.0)
            wps = psum_pool.tile([128, 1024], F32, name="ps", tag="ps")
            for _ in range(56):
                nc.tensor.matmul(
                    wps[0:64, 0:64], scr[:, 0:64], scr[:, 64:128], start=True, stop=True
                )

            accs = [
                acc_pool.tile([128, B], BF16, name=f"acc{ot}", tag=f"acc{ot}")
                for ot in range(OT)
            ]

            def add_op(path, dst, src):
                if path == "V":
                    nc.vector.tensor_add(dst, dst, src)
                elif path == "M":
                    nc.gpsimd.dma_start(dst, src, accum_op=ADD)
                else:
                    nc.gpsimd.tensor_add(dst, dst, src)

            def body(iv=None):
                a_pend = []
                wt_cur = [wt0]
                for nl in range(NL):
                    wt = wt_cur[0]
                    nwt = None
                    for ot in range(OT):
                        ps = psum_pool.tile([128, 1024], F32, name="ps", tag="ps")
                        for ci in range(2):
                            for bt in range(2):
                                nc.tensor.matmul(
                                    ps[:, bt * 512 : bt * 512 + 512],
                                    wt[:, 2 * ci : 2 * ci + 2, ot * 128 : ot * 128 + 128],
                                    st[:, 2 * ci : 2 * ci + 2, bt * 512 : bt * 512 + 512],
                                    start=(ci == 0),
                                    stop=(ci == 1),
                                    perf_mode=DR,
                                )
                        idx = nl * OT + ot
                        acc_t = accs[ot]
                        if _FORM[(nl, ot)] == "D":
                            n_ap = ncbt[:, idx : idx + 1]
                            if nl == 0:
                                nc.vector.tensor_scalar(acc_t[:], ps[:], n_ap, None, op0=MAX)
                            else:
                                nc.vector.scalar_tensor_tensor(
                                    acc_t[:], ps[:], n_ap, acc_t[:], op0=MAX, op1=ADD
                                )
                        else:
                            b_ap = cb[:, idx : idx + 1]
                            if nl == 0:
                                nc.scalar.activation(acc_t[:], ps[:], RELU, bias=b_ap, scale=1.0)
                            else:
                                t = tmp_pool.tile([128, B], BF16, name="tmp", tag=f"tmp{ot % 8}")
                                nc.scalar.activation(t[:], ps[:], RELU, bias=b_ap, scale=1.0)
                                a_pend.append((nl, ot, t))
                        if len(a_pend) >= 3:
                            anl, aot, at = a_pend.pop(0)
                            add_op(_a_add_path(anl, aot, 0), accs[aot][:], at[:])
                        if nl == NL - 1:
                            for item in [x for x in a_pend if x[1] == ot]:
                                a_pend.remove(item)
                                add_op(_a_add_path(item[0], ot, 0), accs[ot][:], item[2][:])
                            nc.sync.dma_start(out_d[:, ot * B : ot * B + B], acc_t[:])
                        if nl < NL - 1 and ot in (2, 6, 10, 14):
                            if ot == 2:
                                nwt = new_wt()
                            wt_chunk_dma(nwt, nl + 1, ot // 4)
                    if nwt is not None:
                        wt_cur[0] = nwt
                for item in a_pend:
                    add_op(_a_add_path(item[0], item[1], 0), accs[item[1]][:], item[2][:])

            if repeat == 1:
                body()
            else:
                with tc.For_i(0, repeat, 1):
                    body()
    nc.compile()
    _cache[repeat] = nc
    return nc


def compute_fold(vertices, W, b):
    v64 = np.asarray(vertices, dtype=np.float64)
    W64 = np.asarray(W, dtype=np.float64)
    b64 = np.asarray(b, dtype=np.float64)
    c = b64 - np.einsum("nd,nod->no", v64, W64)
    fold = np.zeros(OUT, dtype=np.float64)
    for n in range(N):
        nl = n % NL
        for ot in range(OT):
            if _FORM[(nl, ot)] == "D":
                fold[ot * 128 : (ot + 1) * 128] += c[n, ot * 128 : (ot + 1) * 128]
    return fold


def prep_inputs(semantic_vec, vertices, W, b):
    s64 = np.asarray(semantic_vec, dtype=np.float64)
    v64 = np.asarray(vertices, dtype=np.float64)
    W64 = np.asarray(W, dtype=np.float64)
    b64 = np.asarray(b, dtype=np.float64)
    f8 = ml_dtypes.float8_e4m3fn
    c = b64 - np.einsum("nd,nod->no", v64, W64)
    st8 = np.ascontiguousarray(
        s64.reshape(B, DC, 128).transpose(2, 1, 0).reshape(128, DC * B)
    ).astype(f8)
    wt8 = np.ascontiguousarray(
        (AW * W64).reshape(N, NQ, 512, DC, 128).transpose(0, 1, 4, 3, 2).reshape(N, NQ, 128, DC * 512)
    ).astype(f8)
    cbt = np.ascontiguousarray(
        (AW * c).reshape(N_CORES, NL, OT, 128).transpose(0, 3, 1, 2).reshape(N_CORES, 128, NL * OT)
    ).astype(np.float32)
    in_maps = []
    for core in range(N_CORES):
        in_maps.append(
            {"wt": wt8[core * NL : (core + 1) * NL], "st": st8,
             "cb": cbt[core], "ncb": -cbt[core]}
        )
    return in_maps


def kernel(semantic_vec, vertices, W, b):
    nc = build(repeat=1)
    in_maps = prep_inputs(semantic_vec, vertices, W, b)
    fold = compute_fold(vertices, W, b)
    res = run_bass_kernel_spmd(nc, in_maps, core_ids=list(range(N_CORES)))
    total = np.zeros((OUT, B), dtype=np.float32)
    for core in range(N_CORES):
        o = np.asarray(res.results[core]["out"]).astype(np.float32)
        total += o.reshape(128, OT, B).transpose(1, 0, 2).reshape(OUT, B)
    total *= np.float32(1.0 / AW)
    total += fold.astype(np.float32)[:, None]
    return np.ascontiguousarray(total.T)

